# revision 10
# baseline (speedup 1.0000x reference)
"""Criss-cross self-attention on 8 Trainium2 NeuronCores — v7.

Sharding: core = b * 2 + g (b = batch, g = head-group of 4 heads / 256 ch).
Per core, three phases:
  A: qkv = waT.T @ x   (768 rows: q|k|v for the 256 local channels, split as
     two 128-row "p" slices each).  p0's q/k/v stay resident in SBUF; p1's are
     round-tripped through DRAM.
  B: axial attention per (p, d, s) with the transposed-scores formulation:
       scT[l,h] = k_sl^T q_sl          (per 64-ch head, PE)
       eT = exp(scT)                   (one Act op for both heads)
       oT[h, c|den] = eT^T @ [vT|1]    (denominator via ones column)
       on[h,c] = oT * (1/den)          (per-partition scalar, Pool)
       out[c,h] = on^T                 (PE transpose, evicted to out tile)
     d=1 (horizontal) then d=0 (vertical); outputs round-trip DRAM for C.
  C: y = woT.T @ out   (contract over the 512 local attention-out channels).
Host sums the two partial y's per batch element.
"""

import sys

sys.path.insert(0, "/opt/trn_rl_repo")

import numpy as np
import ml_dtypes
import concourse.bass as bass
import concourse.mybir as mybir
from concourse import tile
from concourse.bass_utils import run_bass_kernel_spmd
from concourse.vector_clock import ScopedClock, VectorClock

FP32 = mybir.dt.float32
BF16 = mybir.dt.bfloat16
FP16 = mybir.dt.float16
AF = mybir.ActivationFunctionType

C = 512
H = 128
W = 128
B = 4
S = H * W  # 16384
N_CORES = 8

NBA = 1024  # stage A column block (16 blocks)
NBC = 2048  # stage C column block (8 blocks)

MAX_WAITS = 1
MAX_WAITS_COMPUTE = 1


def _wait_budget(inst):
    tn = type(inst).__name__
    if "DMA" in tn or tn in ("InstNoOp", "InstDrain", "InstCall", "InstHalt"):
        return MAX_WAITS
    return MAX_WAITS_COMPUTE


class PatchedTileContext(tile.TileContext):
    """Work around 'Too many sync wait commands' in this walrus build:
    - the tile tail-drain gets one NOP per outstanding proc
    - any instruction with more than MAX_WAITS sem waits gets the excess
      moved onto same-engine NOPs inserted immediately before it."""

    _nop_seq = 0

    def _split_excess_waits(self, ordered):
        for bb_name, insts in ordered.items():
            out = []
            for inst in insts:
                si = inst.sync_info
                waits = list(si.on_wait) if si is not None and si.on_wait else []
                budget = _wait_budget(inst)
                if len(waits) > budget and inst.engine is not None:
                    keep = waits[:budget]
                    rest = waits[budget:]
                    while rest:
                        chunk, rest = rest[:MAX_WAITS], rest[MAX_WAITS:]
                        PatchedTileContext._nop_seq += 1
                        nop = mybir.InstNoOp(
                            name=f"I-waitsplit-{PatchedTileContext._nop_seq}",
                            ins=[],
                            outs=[],
                        )
                        nop.engine = inst.engine
                        nop.bass_nofuse = True
                        nop.sync_info = mybir.SyncInfo(on_wait=chunk, on_update=[])
                        out.append(nop)
                    inst.sync_info = mybir.SyncInfo(
                        on_wait=keep,
                        on_update=list(si.on_update) if si.on_update else [],
                    )
                out.append(inst)
            ordered[bb_name] = out
        return ordered

    def _lower_ordered_insts(self, ordered):
        super()._lower_ordered_insts(self._split_excess_waits(ordered))

    def _drain_and_barrier(self, tick_clock, wait_clock):
        nc = self.nc
        gc = tick_clock.global_clock
        n = len(gc)
        for proc in range(n):
            t = gc[proc]
            if t > 0:
                nop_inst = nc.sync.nop(nofuse=True)
                vc = VectorClock([t if i == proc else 0 for i in range(n)])
                wait_clock.add_sem_waits(nop_inst.ins, ScopedClock({None: vc}))
        nc.sync.drain()
        nc.all_engine_barrier()
        popped = nc._tile_sem_poison_stack.pop()
        assert popped is self._sem_poison
        nc.clear_and_free_semaphores(list(self.sems.allocated().values()))
        nc.all_engine_barrier()


def build_nc(loop_iters=None, phases="ABC"):
    nc = bass.Bass()
    x = nc.declare_dram_parameter("x", [C, S], FP16, isOutput=False)
    waT = nc.declare_dram_parameter("waT", [C, 768], FP16, isOutput=False)
    woT = nc.declare_dram_parameter("woT", [C, C], FP16, isOutput=False)
    identF = nc.declare_dram_parameter("identF", [128, 128], FP16, isOutput=False)
    identB = nc.declare_dram_parameter("identB", [128, 128], BF16, isOutput=False)
    y = nc.declare_dram_parameter("y", [C, S], FP16, isOutput=True)

    qk1 = nc.dram_tensor("qk1", [256, S], FP16)   # p1 q rows 0:128, k rows 128:256
    v1d = nc.dram_tensor("v1d", [128, S], BF16)   # p1 v
    outd0 = nc.dram_tensor("outd0", [256, S], FP16)  # vertical out (p0, p1)
    outd1 = nc.dram_tensor("outd1", [256, S], FP16)  # horizontal out (p0, p1)

    from contextlib import nullcontext

    with PatchedTileContext(nc) as tc:
        loop_cm = tc.For_i(0, loop_iters, 1) if loop_iters else nullcontext()
        with loop_cm, tc.tile_pool(name="res", bufs=1) as res:
            idF = res.tile([128, 128], FP16, tag="idF")
            nc.sync.dma_start(out=idF[:], in_=identF[:])
            idB = res.tile([128, 128], BF16, tag="idB")
            nc.sync.dma_start(out=idB[:], in_=identB[:])

            q0 = res.tile([128, S], FP16, tag="qres", bufs=1)
            klo0 = res.tile([128, S], FP16, tag="klores", bufs=1)
            khi0 = res.tile([128, S], FP16, tag="khires", bufs=1)
            v0 = res.tile([128, S], BF16, tag="vres", bufs=1)
            ones_res = res.tile([128, 2], BF16, tag="ones")
            nc.gpsimd.memset(ones_res[:], 1.0)
            # zero halves so head matmuls can use full-128 contracts (the
            # unused 64 rows contribute 0); zeroed once, never rewritten
            nc.gpsimd.memset(klo0[64:128, :], 0.0)
            nc.gpsimd.memset(khi0[0:64, :], 0.0)

            # ---------------- stage A ----------------
            if "A" not in phases:
                pass
            else:
              with (
                tc.tile_pool(name="wa", bufs=1) as wap,
                tc.tile_pool(name="xa", bufs=2) as xp,
                tc.tile_pool(name="ao", bufs=2) as aop,
                tc.tile_pool(name="apsum", bufs=4, space="PSUM") as app,
            ):
                wa_t = []
                for k4 in range(4):
                    t = wap.tile([128, 768], FP16, tag=f"wa{k4}")
                    nc.sync.dma_start(out=t[:], in_=waT[k4 * 128:(k4 + 1) * 128, :])
                    wa_t.append(t)
                ev = 0
                for nb in range(S // NBA):
                    c0 = nb * NBA
                    xt = []
                    for k4 in range(4):
                        t = xp.tile([128, NBA], FP16, tag=f"x{k4}")
                        nc.sync.dma_start(
                            out=t[:], in_=x[k4 * 128:(k4 + 1) * 128, c0:c0 + NBA]
                        )
                        xt.append(t)
                    for m in range(6):
                        resident = {0: q0, 4: v0}.get(m)
                        if resident is None:
                            ot = aop.tile(
                                [128, NBA], BF16 if m == 5 else FP16, tag=f"o{m}"
                            )
                        for n2 in range(NBA // 512):
                            ps = app.tile([128, 512], FP32, tag="aps")
                            for k4 in range(4):
                                nc.tensor.matmul(
                                    ps[:],
                                    lhsT=wa_t[k4][:, m * 128:(m + 1) * 128],
                                    rhs=xt[k4][:, n2 * 512:(n2 + 1) * 512],
                                    start=(k4 == 0),
                                    stop=(k4 == 3),
                                )
                            csl2 = slice(c0 + n2 * 512, c0 + (n2 + 1) * 512)
                            if m == 2:
                                # k p0: split into zero-padded variants
                                nc.scalar.copy(
                                    out=klo0[0:64, csl2], in_=ps[0:64, :]
                                )
                                nc.vector.tensor_copy(
                                    khi0[64:128, csl2], ps[64:128, :]
                                )
                            else:
                                if resident is not None:
                                    dst = resident[:, csl2]
                                else:
                                    dst = ot[:, n2 * 512:(n2 + 1) * 512]
                                # gpsimd cannot read PSUM; Act/DVE only
                                if ev % 2 == 0:
                                    nc.scalar.copy(out=dst, in_=ps[:])
                                else:
                                    nc.vector.tensor_copy(dst, ps[:])
                                ev += 1
                        if m == 1:
                            nc.sync.dma_start(out=qk1[0:128, c0:c0 + NBA], in_=ot[:])
                        elif m == 3:
                            nc.sync.dma_start(out=qk1[128:256, c0:c0 + NBA], in_=ot[:])
                        elif m == 5:
                            nc.sync.dma_start(out=v1d[0:128, c0:c0 + NBA], in_=ot[:])

            # ---------------- stage B ----------------
            # B variants: B=full, B2=p0 only (both d), B1=p0+d1 only
            b_ps = (0, 1)
    
    
            b_ds = (1, 0)
            if "B1" in phases:
                b_ps, b_ds = (0,), (1,)
            elif "B2" in phases:
                b_ps = (0,)
            if "B" in phases:
              with (
                tc.tile_pool(name="bwork", bufs=8) as bw,
                tc.tile_pool(name="bout", bufs=2) as bo,
                tc.tile_pool(name="od0p", bufs=1) as odp,
                tc.tile_pool(name="bps", bufs=2, space="PSUM") as bp,
            ):
                it = 0
                # both p's use the same tiles; p1's DMAs overwrite in place
                # (WAR deps via the tile framework), zero halves stay valid
                q_sb, klo_sb, khi_sb, v_sb = q0, klo0, khi0, v0
                for p in b_ps:
                    for d in b_ds:
                        od0 = None
                        if d == 0:
                            od0 = odp.tile([128, S], FP16, tag="od0", bufs=1)

                        def sl(t, lo, hi, s):
                            if d == 0:
                                return t[lo:hi, s::W]
                            return t[lo:hi, s * W:(s + 1) * W]

                        for sp in range(64):
                            s0 = 2 * sp
                            if d == 1 and s0 % 16 == 0:
                                od1 = bo.tile([128, 2048], FP16, tag="od1")
                            # paired transposes of v slices -> one DVE copy
                            tp = bp.tile([128, 256], BF16, tag="tp", bufs=1)
                            for j in (0, 1):
                                nc.tensor.transpose(
                                    tp[:, j * 128:(j + 1) * 128],
                                    sl(v_sb, 0, 128, s0 + j),
                                    idB[:],
                                )
                            vt = bw.tile([128, 256], BF16, tag="vt")
                            nc.vector.tensor_copy(vt[:, 0:128], tp[:, 0:128])
                            nc.scalar.copy(out=vt[:, 128:256], in_=tp[:, 128:256])
                            # head0 scores for both s in one bank -> one exp
                            # (base-0 operands may write sliced PSUM APs;
                            # base-64 head1 must write whole [128,128] tiles)
                            eT = bw.tile([128, 512], BF16, tag="eT")
                            # head scores via zero-padded 128-contract
                            # stationaries: base-0 operands allow sliced
                            # PSUM writes and pair-batched exps
                            for h, ksrc in ((0, klo_sb), (1, khi_sb)):
                                sch = bp.tile(
                                    [128, 256], FP32, tag=f"scp{h}", bufs=2
                                )
                                for j in (0, 1):
                                    nc.tensor.matmul(
                                        sch[:, j * 128:(j + 1) * 128],
                                        lhsT=sl(ksrc, 0, 128, s0 + j),
                                        rhs=sl(q_sb, 0, 128, s0 + j),
                                        start=True,
                                        stop=True,
                                    )
                                nc.scalar.activation(
                                    eT[:, h * 256:(h + 1) * 256], sch[:], AF.Exp
                                )
                            # eT: h0s0|h0s1|h1s0|h1s1 ; vt: s0(ch0:128)|s1
                            # oT: outs s-major 0:256 (j*128+h*64), dens
                            # contiguous 256:260 at col 256 + j*2 + h
                            oT = bp.tile([128, 260], FP32, tag="oT")
                            for j in (0, 1):
                                for h in (0, 1):
                                    lh = eT[:, h * 256 + j * 128: h * 256 + j * 128 + 128]
                                    nc.tensor.matmul(
                                        oT[:, j * 128 + h * 64: j * 128 + (h + 1) * 64],
                                        lhsT=lh,
                                        rhs=vt[:, j * 128 + h * 64: j * 128 + (h + 1) * 64],
                                        start=True,
                                        stop=True,
                                    )
                                    nc.tensor.matmul(
                                        oT[:, 256 + j * 2 + h: 257 + j * 2 + h],
                                        lhsT=lh,
                                        rhs=ones_res[:, 0:1],
                                        start=True,
                                        stop=True,
                                    )
                            rec = bw.tile([128, 4], FP32, tag="rec")
                            nc.vector.reciprocal(rec[:], oT[:, 256:260])
                            on = bw.tile([128, 256], FP16, tag="on")
                            for j in (0, 1):
                                nc.scalar.activation(
                                    on[:, j * 128:j * 128 + 64],
                                    oT[:, j * 128:j * 128 + 64],
                                    AF.Identity,
                                    scale=rec[:, j * 2:j * 2 + 1],
                                )
                                nc.vector.tensor_scalar_mul(
                                    on[:, j * 128 + 64:(j + 1) * 128],
                                    oT[:, j * 128 + 64:(j + 1) * 128],
                                    rec[:, j * 2 + 1:j * 2 + 2],
                                )
                            ft = bp.tile([128, 256], FP16, tag="ft", bufs=1)
                            for j in (0, 1):
                                nc.tensor.transpose(
                                    ft[:, j * 128:(j + 1) * 128],
                                    on[:, j * 128:(j + 1) * 128],
                                    idF[:],
                                )
                            if d == 1:
                                cc = (s0 % 16) * 128
                                if sp % 4 == 0:
                                    nc.vector.tensor_copy(
                                        od1[:, cc:cc + 256], ft[:]
                                    )
                                else:
                                    nc.scalar.copy(
                                        out=od1[:, cc:cc + 256], in_=ft[:]
                                    )
                                if s0 % 16 == 14:
                                    c2 = (s0 // 16) * 2048
                                    nc.sync.dma_start(
                                        out=outd1[p * 128:(p + 1) * 128, c2:c2 + 2048],
                                        in_=od1[:],
                                    )
                            else:
                                for j in (0, 1):
                                    src_ap = ft[:, j * 128:(j + 1) * 128]
                                    if (sp + j) % 2 == 0:
                                        nc.vector.tensor_copy(
                                            od0[:, s0 + j::W], src_ap
                                        )
                                    else:
                                        nc.scalar.copy(
                                            out=od0[:, s0 + j::W], in_=src_ap
                                        )
                            it += 1
                        if d == 0:
                            nc.sync.dma_start(
                                out=outd0[p * 128:(p + 1) * 128, :], in_=od0[:]
                            )
                    if p == 0 and len(b_ps) > 1:
                        nc.sync.dma_start(out=q0[:], in_=qk1[0:128, :])
                        nc.sync.dma_start(
                            out=klo0[0:64, :], in_=qk1[128:192, :]
                        )
                        nc.sync.dma_start(
                            out=khi0[64:128, :], in_=qk1[192:256, :]
                        )
                        nc.sync.dma_start(out=v0[:], in_=v1d[0:128, :])

            # ---------------- stage C ----------------
            if "C" in phases:
              with (
                tc.tile_pool(name="wo", bufs=1) as wop,
                tc.tile_pool(name="cr", bufs=2) as crp,
                tc.tile_pool(name="cy", bufs=2) as cyp,
                tc.tile_pool(name="cpsum", bufs=4, space="PSUM") as cpp,
            ):
                wo_t = []
                for k4 in range(4):
                    t = wop.tile([128, 512], FP16, tag=f"wo{k4}")
                    nc.sync.dma_start(out=t[:], in_=woT[k4 * 128:(k4 + 1) * 128, :])
                    wo_t.append(t)
                srcs = [
                    (outd0, 0), (outd0, 128), (outd1, 0), (outd1, 128),
                ]
                ev = 0
                for nb in range(S // NBC):
                    c0 = nb * NBC
                    rt = []
                    for k4, (srct, r0) in enumerate(srcs):
                        t = crp.tile([128, NBC], FP16, tag=f"r{k4}")
                        nc.sync.dma_start(
                            out=t[:], in_=srct[r0:r0 + 128, c0:c0 + NBC]
                        )
                        rt.append(t)
                    for m in range(4):
                        yt = cyp.tile([128, NBC], FP16, tag="yt")
                        for n2 in range(NBC // 512):
                            ps = cpp.tile([128, 512], FP32, tag="cps")
                            for k4 in range(4):
                                nc.tensor.matmul(
                                    ps[:],
                                    lhsT=wo_t[k4][:, m * 128:(m + 1) * 128],
                                    rhs=rt[k4][:, n2 * 512:(n2 + 1) * 512],
                                    start=(k4 == 0),
                                    stop=(k4 == 3),
                                )
                            dst = yt[:, n2 * 512:(n2 + 1) * 512]
                            if ev % 2 == 0:
                                nc.scalar.copy(out=dst, in_=ps[:])
                            else:
                                nc.vector.tensor_copy(dst, ps[:])
                            ev += 1
                        nc.sync.dma_start(
                            out=y[m * 128:(m + 1) * 128, c0:c0 + NBC], in_=yt[:]
                        )
    return nc


def make_in_maps(x, Wq, Wk, Wv, Wo):
    x = np.asarray(x, dtype=np.float32).reshape(B, C, S)
    Wq = np.asarray(Wq, np.float32)
    Wk = np.asarray(Wk, np.float32)
    Wv = np.asarray(Wv, np.float32)
    Wo = np.asarray(Wo, np.float32)
    identF = np.eye(128, dtype=np.float16)
    identB = np.eye(128).astype(ml_dtypes.bfloat16)

    def f16(a):
        return np.ascontiguousarray(a).astype(np.float16)

    in_maps = []
    for core in range(N_CORES):
        b, g = divmod(core, 2)
        lo, hi = g * 256, (g + 1) * 256
        wa = np.concatenate([Wq[lo:hi], Wk[lo:hi], Wv[lo:hi]], axis=0).T.copy()
        wo_loc = np.concatenate(
            [Wo[:, lo:hi].T, Wo[:, C + lo: C + hi].T], axis=0
        ).copy()
        in_maps.append(
            {
                "x": f16(x[b]),
                "waT": f16(wa),
                "woT": f16(wo_loc),
                "identF": identF,
                "identB": identB,
            }
        )
    return in_maps


def combine_results(results):
    y = np.empty((B, C, H, W), np.float32)
    for b in range(B):
        y[b] = (
            results[2 * b]["y"].astype(np.float32)
            + results[2 * b + 1]["y"].astype(np.float32)
        ).reshape(C, H, W)
    return y


_NC_CACHE = None


def get_nc():
    global _NC_CACHE
    if _NC_CACHE is None:
        _NC_CACHE = build_nc()
    return _NC_CACHE


def kernel(x, Wq, Wk, Wv, Wo):
    nc = get_nc()
    in_maps = make_in_maps(x, Wq, Wk, Wv, Wo)
    res = run_bass_kernel_spmd(nc, in_maps, list(range(N_CORES)), trace=False)
    return combine_results(res.results)


# revision 11
# speedup vs baseline: 1.2534x; 1.2534x over previous
"""Criss-cross self-attention on 8 Trainium2 NeuronCores — v5.

Sharding: core = b * 2 + g (b = batch, g = head-group of 4 heads / 256 ch).
Per core, three phases:
  A: qkv = waT.T @ x   (768 rows: q|k|v for the 256 local channels, split as
     two 128-row "p" slices each).  p0's q/k/v stay resident in SBUF; p1's are
     round-tripped through DRAM.
  B: axial attention per (p, d, s) with the transposed-scores formulation:
       scT[l,h] = k_sl^T q_sl          (per 64-ch head, PE)
       eT = exp(scT)                   (one Act op for both heads)
       oT[h, c|den] = eT^T @ [vT|1]    (denominator via ones column)
       on[h,c] = oT * (1/den)          (per-partition scalar, Pool)
       out[c,h] = on^T                 (PE transpose, evicted to out tile)
     d=1 (horizontal) then d=0 (vertical); outputs round-trip DRAM for C.
  C: y = woT.T @ out   (contract over the 512 local attention-out channels).
Host sums the two partial y's per batch element.
"""

import sys

sys.path.insert(0, "/opt/trn_rl_repo")

import numpy as np
import ml_dtypes
import concourse.bass as bass
import concourse.mybir as mybir
from concourse import tile
from concourse.bass_utils import run_bass_kernel_spmd
from concourse.vector_clock import ScopedClock, VectorClock

FP32 = mybir.dt.float32
BF16 = mybir.dt.bfloat16
FP16 = mybir.dt.float16
AF = mybir.ActivationFunctionType

C = 512
H = 128
W = 128
B = 4
S = H * W  # 16384
N_CORES = 8

NBA = 1024  # stage A column block (16 blocks)
NBC = 2048  # stage C column block (8 blocks)

MAX_WAITS = 1
MAX_WAITS_COMPUTE = 1


def _wait_budget(inst):
    tn = type(inst).__name__
    if "DMA" in tn or tn in ("InstNoOp", "InstDrain", "InstCall", "InstHalt"):
        return MAX_WAITS
    return MAX_WAITS_COMPUTE


class PatchedTileContext(tile.TileContext):
    """Work around 'Too many sync wait commands' in this walrus build:
    - the tile tail-drain gets one NOP per outstanding proc
    - any instruction with more than MAX_WAITS sem waits gets the excess
      moved onto same-engine NOPs inserted immediately before it."""

    _nop_seq = 0

    def _split_excess_waits(self, ordered):
        for bb_name, insts in ordered.items():
            out = []
            for inst in insts:
                si = inst.sync_info
                waits = list(si.on_wait) if si is not None and si.on_wait else []
                budget = _wait_budget(inst)
                if len(waits) > budget and inst.engine is not None:
                    keep = waits[:budget]
                    rest = waits[budget:]
                    while rest:
                        chunk, rest = rest[:MAX_WAITS], rest[MAX_WAITS:]
                        PatchedTileContext._nop_seq += 1
                        nop = mybir.InstNoOp(
                            name=f"I-waitsplit-{PatchedTileContext._nop_seq}",
                            ins=[],
                            outs=[],
                        )
                        nop.engine = inst.engine
                        nop.bass_nofuse = True
                        nop.sync_info = mybir.SyncInfo(on_wait=chunk, on_update=[])
                        out.append(nop)
                    inst.sync_info = mybir.SyncInfo(
                        on_wait=keep,
                        on_update=list(si.on_update) if si.on_update else [],
                    )
                out.append(inst)
            ordered[bb_name] = out
        return ordered

    def _lower_ordered_insts(self, ordered):
        super()._lower_ordered_insts(self._split_excess_waits(ordered))

    def _drain_and_barrier(self, tick_clock, wait_clock):
        nc = self.nc
        gc = tick_clock.global_clock
        n = len(gc)
        for proc in range(n):
            t = gc[proc]
            if t > 0:
                nop_inst = nc.sync.nop(nofuse=True)
                vc = VectorClock([t if i == proc else 0 for i in range(n)])
                wait_clock.add_sem_waits(nop_inst.ins, ScopedClock({None: vc}))
        nc.sync.drain()
        nc.all_engine_barrier()
        popped = nc._tile_sem_poison_stack.pop()
        assert popped is self._sem_poison
        nc.clear_and_free_semaphores(list(self.sems.allocated().values()))
        nc.all_engine_barrier()


def build_nc(loop_iters=None, phases="ABC"):
    nc = bass.Bass()
    x = nc.declare_dram_parameter("x", [C, S], FP16, isOutput=False)
    waT = nc.declare_dram_parameter("waT", [C, 768], FP16, isOutput=False)
    woT = nc.declare_dram_parameter("woT", [C, C], FP16, isOutput=False)
    identF = nc.declare_dram_parameter("identF", [128, 128], FP16, isOutput=False)
    identB = nc.declare_dram_parameter("identB", [128, 128], BF16, isOutput=False)
    y = nc.declare_dram_parameter("y", [C, S], FP16, isOutput=True)

    qk1 = nc.dram_tensor("qk1", [256, S], FP16)   # p1 q rows 0:128, k rows 128:256
    v1d = nc.dram_tensor("v1d", [128, S], BF16)   # p1 v
    outd0 = nc.dram_tensor("outd0", [256, S], FP16)  # vertical out (p0, p1)
    outd1 = nc.dram_tensor("outd1", [256, S], FP16)  # horizontal out (p0, p1)

    from contextlib import nullcontext

    with PatchedTileContext(nc) as tc:
        loop_cm = tc.For_i(0, loop_iters, 1) if loop_iters else nullcontext()
        with loop_cm, tc.tile_pool(name="res", bufs=1) as res:
            idF = res.tile([128, 128], FP16, tag="idF")
            nc.sync.dma_start(out=idF[:], in_=identF[:])
            idB = res.tile([128, 128], BF16, tag="idB")
            nc.sync.dma_start(out=idB[:], in_=identB[:])

            q0 = res.tile([128, S], FP16, tag="qres", bufs=1)
            klo0 = res.tile([128, S], FP16, tag="klores", bufs=1)
            khi0 = res.tile([128, S], FP16, tag="khires", bufs=1)
            v0 = res.tile([128, S], BF16, tag="vres", bufs=1)
            ones_res = res.tile([128, 2], BF16, tag="ones")
            nc.gpsimd.memset(ones_res[:], 1.0)
            # zero halves so head matmuls can use full-128 contracts (the
            # unused 64 rows contribute 0); zeroed once, never rewritten
            nc.gpsimd.memset(klo0[64:128, :], 0.0)
            nc.gpsimd.memset(khi0[0:64, :], 0.0)

            # ---------------- stage A ----------------
            if "A" not in phases:
                pass
            else:
              with (
                tc.tile_pool(name="wa", bufs=1) as wap,
                tc.tile_pool(name="xa", bufs=2) as xp,
                tc.tile_pool(name="ao", bufs=2) as aop,
                tc.tile_pool(name="apsum", bufs=4, space="PSUM") as app,
            ):
                wa_t = []
                for k4 in range(4):
                    t = wap.tile([128, 768], FP16, tag=f"wa{k4}")
                    nc.sync.dma_start(out=t[:], in_=waT[k4 * 128:(k4 + 1) * 128, :])
                    wa_t.append(t)
                ev = 0
                for nb in range(S // NBA):
                    c0 = nb * NBA
                    xt = []
                    for k4 in range(4):
                        t = xp.tile([128, NBA], FP16, tag=f"x{k4}")
                        nc.sync.dma_start(
                            out=t[:], in_=x[k4 * 128:(k4 + 1) * 128, c0:c0 + NBA]
                        )
                        xt.append(t)
                    for m in range(6):
                        resident = {0: q0, 4: v0}.get(m)
                        if resident is None:
                            ot = aop.tile(
                                [128, NBA], BF16 if m == 5 else FP16, tag=f"o{m}"
                            )
                        for n2 in range(NBA // 512):
                            ps = app.tile([128, 512], FP32, tag="aps")
                            for k4 in range(4):
                                nc.tensor.matmul(
                                    ps[:],
                                    lhsT=wa_t[k4][:, m * 128:(m + 1) * 128],
                                    rhs=xt[k4][:, n2 * 512:(n2 + 1) * 512],
                                    start=(k4 == 0),
                                    stop=(k4 == 3),
                                )
                            csl2 = slice(c0 + n2 * 512, c0 + (n2 + 1) * 512)
                            if m == 2:
                                # k p0: split into zero-padded variants
                                nc.scalar.copy(
                                    out=klo0[0:64, csl2], in_=ps[0:64, :]
                                )
                                nc.vector.tensor_copy(
                                    khi0[64:128, csl2], ps[64:128, :]
                                )
                            else:
                                if resident is not None:
                                    dst = resident[:, csl2]
                                else:
                                    dst = ot[:, n2 * 512:(n2 + 1) * 512]
                                # gpsimd cannot read PSUM; Act/DVE only
                                if ev % 2 == 0:
                                    nc.scalar.copy(out=dst, in_=ps[:])
                                else:
                                    nc.vector.tensor_copy(dst, ps[:])
                                ev += 1
                        if m == 1:
                            nc.sync.dma_start(out=qk1[0:128, c0:c0 + NBA], in_=ot[:])
                        elif m == 3:
                            nc.sync.dma_start(out=qk1[128:256, c0:c0 + NBA], in_=ot[:])
                        elif m == 5:
                            nc.sync.dma_start(out=v1d[0:128, c0:c0 + NBA], in_=ot[:])

            # ---------------- stage B ----------------
            # B variants: B=full, B2=p0 only (both d), B1=p0+d1 only
            b_ps = (0, 1)
    
    
            b_ds = (1, 0)
            if "B1" in phases:
                b_ps, b_ds = (0,), (1,)
            elif "B2" in phases:
                b_ps = (0,)
            if "B" in phases:
              with (
                tc.tile_pool(name="bwork", bufs=6) as bw,
                tc.tile_pool(name="bout", bufs=2) as bo,
                tc.tile_pool(name="od0p", bufs=1) as odp,
                tc.tile_pool(name="bps", bufs=2, space="PSUM") as bp,
            ):
                it = 0
                # both p's use the same tiles; p1's DMAs overwrite in place
                # (WAR deps via the tile framework), zero halves stay valid
                q_sb, klo_sb, khi_sb, v_sb = q0, klo0, khi0, v0
                for p in b_ps:
                    for d in b_ds:
                        od0 = None
                        if d == 0:
                            od0 = odp.tile([128, S], FP16, tag="od0", bufs=1)

                        def sl(t, lo, hi, s):
                            if d == 0:
                                return t[lo:hi, s::W]
                            return t[lo:hi, s * W:(s + 1) * W]

                        for sp in range(64):
                            s0 = 2 * sp
                            if d == 1 and s0 % 16 == 0:
                                od1 = bo.tile([128, 2048], FP16, tag="od1")
                            # paired transposes of v slices -> one DVE copy
                            tp = bp.tile([128, 256], BF16, tag="tp", bufs=1)
                            for j in (0, 1):
                                nc.tensor.transpose(
                                    tp[:, j * 128:(j + 1) * 128],
                                    sl(v_sb, 0, 128, s0 + j),
                                    idB[:],
                                )
                            vt = bw.tile([128, 256], BF16, tag="vt")
                            nc.vector.tensor_copy(vt[:, 0:128], tp[:, 0:128])
                            nc.scalar.copy(out=vt[:, 128:256], in_=tp[:, 128:256])
                            # head0 scores for both s in one bank -> one exp
                            # (base-0 operands may write sliced PSUM APs;
                            # base-64 head1 must write whole [128,128] tiles)
                            eT = bw.tile([128, 512], BF16, tag="eT")
                            # head scores via zero-padded 128-contract
                            # stationaries: base-0 operands allow sliced
                            # PSUM writes and pair-batched exps
                            for h, ksrc in ((0, klo_sb), (1, khi_sb)):
                                sch = bp.tile(
                                    [128, 256], FP32, tag=f"scp{h}", bufs=2
                                )
                                for j in (0, 1):
                                    nc.tensor.matmul(
                                        sch[:, j * 128:(j + 1) * 128],
                                        lhsT=sl(ksrc, 0, 128, s0 + j),
                                        rhs=sl(q_sb, 0, 128, s0 + j),
                                        start=True,
                                        stop=True,
                                    )
                                nc.scalar.activation(
                                    eT[:, h * 256:(h + 1) * 256], sch[:], AF.Exp
                                )
                            # eT: h0s0|h0s1|h1s0|h1s1 ; vt: s0(ch0:128)|s1
                            # oT: outs s-major 0:256 (j*128+h*64), dens
                            # contiguous 256:260 at col 256 + j*2 + h
                            oT = bp.tile([128, 260], FP32, tag="oT")
                            for j in (0, 1):
                                for h in (0, 1):
                                    lh = eT[:, h * 256 + j * 128: h * 256 + j * 128 + 128]
                                    nc.tensor.matmul(
                                        oT[:, j * 128 + h * 64: j * 128 + (h + 1) * 64],
                                        lhsT=lh,
                                        rhs=vt[:, j * 128 + h * 64: j * 128 + (h + 1) * 64],
                                        start=True,
                                        stop=True,
                                    )
                                    nc.tensor.matmul(
                                        oT[:, 256 + j * 2 + h: 257 + j * 2 + h],
                                        lhsT=lh,
                                        rhs=ones_res[:, 0:1],
                                        start=True,
                                        stop=True,
                                    )
                            rec = bw.tile([128, 4], FP32, tag="rec")
                            nc.vector.reciprocal(rec[:], oT[:, 256:260])
                            on = bw.tile([128, 256], FP16, tag="on")
                            for j in (0, 1):
                                nc.scalar.activation(
                                    on[:, j * 128:j * 128 + 64],
                                    oT[:, j * 128:j * 128 + 64],
                                    AF.Identity,
                                    scale=rec[:, j * 2:j * 2 + 1],
                                )
                                nc.vector.tensor_scalar_mul(
                                    on[:, j * 128 + 64:(j + 1) * 128],
                                    oT[:, j * 128 + 64:(j + 1) * 128],
                                    rec[:, j * 2 + 1:j * 2 + 2],
                                )
                            ft = bp.tile([128, 256], FP16, tag="ft", bufs=1)
                            for j in (0, 1):
                                nc.tensor.transpose(
                                    ft[:, j * 128:(j + 1) * 128],
                                    on[:, j * 128:(j + 1) * 128],
                                    idF[:],
                                )
                            if d == 1:
                                cc = (s0 % 16) * 128
                                if sp % 2 == 0:
                                    nc.vector.tensor_copy(
                                        od1[:, cc:cc + 256], ft[:]
                                    )
                                else:
                                    nc.scalar.copy(
                                        out=od1[:, cc:cc + 256], in_=ft[:]
                                    )
                                if s0 % 16 == 14:
                                    c2 = (s0 // 16) * 2048
                                    nc.sync.dma_start(
                                        out=outd1[p * 128:(p + 1) * 128, c2:c2 + 2048],
                                        in_=od1[:],
                                    )
                            else:
                                for j in (0, 1):
                                    src_ap = ft[:, j * 128:(j + 1) * 128]
                                    if (sp + j) % 2 == 0:
                                        nc.vector.tensor_copy(
                                            od0[:, s0 + j::W], src_ap
                                        )
                                    else:
                                        nc.scalar.copy(
                                            out=od0[:, s0 + j::W], in_=src_ap
                                        )
                            it += 1
                        if d == 0:
                            nc.sync.dma_start(
                                out=outd0[p * 128:(p + 1) * 128, :], in_=od0[:]
                            )
                    if p == 0 and len(b_ps) > 1:
                        nc.sync.dma_start(out=q0[:], in_=qk1[0:128, :])
                        nc.sync.dma_start(
                            out=klo0[0:64, :], in_=qk1[128:192, :]
                        )
                        nc.sync.dma_start(
                            out=khi0[64:128, :], in_=qk1[192:256, :]
                        )
                        nc.sync.dma_start(out=v0[:], in_=v1d[0:128, :])

            # ---------------- stage C ----------------
            if "C" in phases:
              with (
                tc.tile_pool(name="wo", bufs=1) as wop,
                tc.tile_pool(name="cr", bufs=2) as crp,
                tc.tile_pool(name="cy", bufs=2) as cyp,
                tc.tile_pool(name="cpsum", bufs=4, space="PSUM") as cpp,
            ):
                wo_t = []
                for k4 in range(4):
                    t = wop.tile([128, 512], FP16, tag=f"wo{k4}")
                    nc.sync.dma_start(out=t[:], in_=woT[k4 * 128:(k4 + 1) * 128, :])
                    wo_t.append(t)
                srcs = [
                    (outd0, 0), (outd0, 128), (outd1, 0), (outd1, 128),
                ]
                ev = 0
                for nb in range(S // NBC):
                    c0 = nb * NBC
                    rt = []
                    for k4, (srct, r0) in enumerate(srcs):
                        t = crp.tile([128, NBC], FP16, tag=f"r{k4}")
                        nc.sync.dma_start(
                            out=t[:], in_=srct[r0:r0 + 128, c0:c0 + NBC]
                        )
                        rt.append(t)
                    for m in range(4):
                        yt = cyp.tile([128, NBC], FP16, tag="yt")
                        for n2 in range(NBC // 512):
                            ps = cpp.tile([128, 512], FP32, tag="cps")
                            for k4 in range(4):
                                nc.tensor.matmul(
                                    ps[:],
                                    lhsT=wo_t[k4][:, m * 128:(m + 1) * 128],
                                    rhs=rt[k4][:, n2 * 512:(n2 + 1) * 512],
                                    start=(k4 == 0),
                                    stop=(k4 == 3),
                                )
                            dst = yt[:, n2 * 512:(n2 + 1) * 512]
                            if ev % 2 == 0:
                                nc.scalar.copy(out=dst, in_=ps[:])
                            else:
                                nc.vector.tensor_copy(dst, ps[:])
                            ev += 1
                        nc.sync.dma_start(
                            out=y[m * 128:(m + 1) * 128, c0:c0 + NBC], in_=yt[:]
                        )
    return nc


def make_in_maps(x, Wq, Wk, Wv, Wo):
    x = np.asarray(x, dtype=np.float32).reshape(B, C, S)
    Wq = np.asarray(Wq, np.float32)
    Wk = np.asarray(Wk, np.float32)
    Wv = np.asarray(Wv, np.float32)
    Wo = np.asarray(Wo, np.float32)
    identF = np.eye(128, dtype=np.float16)
    identB = np.eye(128).astype(ml_dtypes.bfloat16)

    def f16(a):
        return np.ascontiguousarray(a).astype(np.float16)

    in_maps = []
    for core in range(N_CORES):
        b, g = divmod(core, 2)
        lo, hi = g * 256, (g + 1) * 256
        wa = np.concatenate([Wq[lo:hi], Wk[lo:hi], Wv[lo:hi]], axis=0).T.copy()
        wo_loc = np.concatenate(
            [Wo[:, lo:hi].T, Wo[:, C + lo: C + hi].T], axis=0
        ).copy()
        in_maps.append(
            {
                "x": f16(x[b]),
                "waT": f16(wa),
                "woT": f16(wo_loc),
                "identF": identF,
                "identB": identB,
            }
        )
    return in_maps


def combine_results(results):
    y = np.empty((B, C, H, W), np.float32)
    for b in range(B):
        y[b] = (
            results[2 * b]["y"].astype(np.float32)
            + results[2 * b + 1]["y"].astype(np.float32)
        ).reshape(C, H, W)
    return y


_NC_CACHE = None


def get_nc():
    global _NC_CACHE
    if _NC_CACHE is None:
        _NC_CACHE = build_nc()
    return _NC_CACHE


def kernel(x, Wq, Wk, Wv, Wo):
    nc = get_nc()
    in_maps = make_in_maps(x, Wq, Wk, Wv, Wo)
    res = run_bass_kernel_spmd(nc, in_maps, list(range(N_CORES)), trace=False)
    return combine_results(res.results)
